# revision 5
# baseline (speedup 1.0000x reference)
"""Trainium2 Bass kernel for nn_Damping (B=32768, N=64, H=256).

Per-sample computation:
    diag = (relu(MLP_d(x)) + damp_min) * x          # [64]
    off  = MLP_o(x)                                  # [2016] strictly-lower entries
    L    = scatter(off -> strict lower, diag -> diagonal)   # [64, 64]
    out  = L @ (L^T @ x)

Strategy: pure data parallel over 8 NeuronCores (4096 samples each).
On-chip layout is feature-major ("transposed"): activations live as
[features(partitions), batch(free)] tiles of 512 samples. The scatter
matvecs are computed without materializing L:
    v   = Ecol^T @ (off ⊙ (Rrow @ xT)) + diag ⊙ x       (v = L^T x)
    out = Erow^T @ (off ⊙ (Rcol @ vT)) + diag ⊙ v       (out = L v)
where Rrow/Rcol are 0/1 expansion matrices (PE matmuls) and Ecol/Erow are
0/1 reduction matrices (PE matmuls accumulating in PSUM fp32). Matmul
operands are bf16 (full PE rate + fast weight load); accumulation and the
diag path stay fp32. The off dimension is zero-padded 2016 -> 2048 so all
weight slices are uniform 128 columns.
"""

import numpy as np

B, N, H, OFF = 32768, 64, 256, 2016
NCORES = 8
BLOCAL = B // NCORES          # 4096 samples per core
NSLICES = 16
SL = 128                      # padded slice width; 16*128 = 2048
OFFP = NSLICES * SL           # 2048 (padded off dim)
NBLOCKS = 8                   # blocks of 512 samples per core
BT = 512                      # batch tile (moving free dim)

_compiled = None


def _build_program():
    import concourse.bass as bass  # noqa: F401
    import concourse.mybir as mybir
    import concourse.tile as tile
    from concourse import bacc
    from concourse.masks import make_identity

    f32 = mybir.dt.float32
    bf16 = mybir.dt.bfloat16
    AF = mybir.ActivationFunctionType

    nc = bacc.Bacc("TRN2", target_bir_lowering=False, debug=False,
                   num_devices=NCORES)

    def din(name, shape, dt=f32):
        return nc.dram_tensor(name, list(shape), dt, kind="ExternalInput").ap()

    x_ap = din("x", (BLOCAL, N))
    wd1_ap = din("wd1", (N, H), bf16)
    wd2_ap = din("wd2", (128, 2, H), bf16)
    wdo_ap = din("wdo", (128, 2, N), bf16)
    wo1_ap = din("wo1", (N, H), bf16)
    wo2_ap = din("wo2", (128, 2, H), bf16)
    woo_ap = din("woo", (128, 2, OFFP), bf16)
    bd1_ap = din("bd1", (128, 2))
    bd2_ap = din("bd2", (128, 2))
    bo1_ap = din("bo1", (128, 2))
    bo2_ap = din("bo2", (128, 2))
    bdo_ap = din("bdo", (N, 1))
    boo_ap = din("boo", (SL, NSLICES))
    dm_ap = din("dm", (N, 1))
    rrow_ap = din("rrow", (N, OFFP), bf16)
    rcol_ap = din("rcol", (N, OFFP), bf16)
    ecol_ap = din("ecol", (SL, NSLICES * N), bf16)
    erow_ap = din("erow", (SL, NSLICES * N), bf16)
    out_ap = nc.dram_tensor("out", [BLOCAL, N], f32, kind="ExternalOutput").ap()

    # view: partition p holds samples [32p, 32p+32); block b covers q in [4b,4b+4)
    x_view = x_ap.rearrange("(p q) n -> p (q n)", p=128)       # [128, 2048]
    out_view = out_ap.rearrange("(p q) n -> p q n", p=128)     # [128, 32, 64]

    with tile.TileContext(nc) as tc:
        with (
            tc.tile_pool(name="consts", bufs=1) as consts,
            tc.tile_pool(name="xt", bufs=2) as xt_pool,
            tc.tile_pool(name="acts", bufs=2) as act_pool,
            tc.tile_pool(name="offp", bufs=2) as off_pool,
            tc.tile_pool(name="mp", bufs=3) as m_pool,
            tc.tile_pool(name="small", bufs=2) as small_pool,
            tc.tile_pool(name="outp", bufs=2) as out_pool,
            tc.tile_pool(name="ps_mlp", bufs=2, space="PSUM") as ps_mlp,
            tc.tile_pool(name="ps_off", bufs=2, space="PSUM") as ps_off,
            tc.tile_pool(name="ps_xe", bufs=3, space="PSUM") as ps_xe,
            tc.tile_pool(name="ps_acc", bufs=1, space="PSUM") as ps_acc,
        ):
            # ---- load constants ----
            def load(name, shape, ap):
                t = consts.tile(list(shape), ap.dtype, tag=name)
                nc.sync.dma_start(t[:], ap)
                return t

            wd1 = load("wd1", (N, H), wd1_ap)
            wd2 = load("wd2", (128, 2, H), wd2_ap)
            wdo = load("wdo", (128, 2, N), wdo_ap)
            wo1 = load("wo1", (N, H), wo1_ap)
            wo2 = load("wo2", (128, 2, H), wo2_ap)
            woo = load("woo", (128, 2, OFFP), woo_ap)
            bd1 = load("bd1", (128, 2), bd1_ap)
            bd2 = load("bd2", (128, 2), bd2_ap)
            bo1 = load("bo1", (128, 2), bo1_ap)
            bo2 = load("bo2", (128, 2), bo2_ap)
            bdo = load("bdo", (N, 1), bdo_ap)
            boo = load("boo", (SL, NSLICES), boo_ap)
            dm = load("dm", (N, 1), dm_ap)
            rrow = load("rrow", (N, OFFP), rrow_ap)
            rcol = load("rcol", (N, OFFP), rcol_ap)
            ecol = load("ecol", (SL, NSLICES * N), ecol_ap)
            erow = load("erow", (SL, NSLICES * N), erow_ap)
            xfull32 = load("xfull32", (128, NBLOCKS * 4 * N), x_view)

            xfull = consts.tile([128, NBLOCKS * 4 * N], bf16, tag="xfull")
            nc.vector.tensor_copy(xfull[:], xfull32[:])

            identb = consts.tile([128, 128], bf16, tag="identb")
            make_identity(nc, identb[:])
            identf = consts.tile([64, 64], f32, tag="identf")
            make_identity(nc, identf[:])

            def mlp2(w1, b1, w2, b2, xT, tag):
                """Two tanh layers; returns [128, 2, 512] feature-major bf16."""
                a1 = act_pool.tile([128, 2, BT], bf16, tag=tag + "1")
                for s in range(2):
                    ps = ps_mlp.tile([128, BT], f32, tag="mlp")
                    nc.tensor.matmul(ps[:], w1[:, 128 * s:128 * (s + 1)],
                                     xT[:], start=True, stop=True)
                    nc.scalar.activation(a1[:, s], ps[:], AF.Tanh,
                                         bias=b1[:, s:s + 1])
                a2 = act_pool.tile([128, 2, BT], bf16, tag=tag + "2")
                for s in range(2):
                    ps = ps_mlp.tile([128, BT], f32, tag="mlp")
                    for k in range(2):
                        nc.tensor.matmul(ps[:], w2[:, k, 128 * s:128 * (s + 1)],
                                         a1[:, k], start=(k == 0), stop=(k == 1))
                    nc.scalar.activation(a2[:, s], ps[:], AF.Tanh,
                                         bias=b2[:, s:s + 1])
                return a2

            for b in range(NBLOCKS):
                # ---- transpose x block to feature-major [64, 512] bf16 ----
                xT = xt_pool.tile([N, BT], bf16, tag="xT")
                for t in range(4):
                    pst = ps_xe.tile([N, 128], bf16, tag="xe")
                    nc.tensor.transpose(
                        pst[:], xfull[:, (4 * b + t) * N:(4 * b + t + 1) * N],
                        identb[:])
                    nc.scalar.copy(xT[:, 128 * t:128 * (t + 1)], pst[:])

                # ---- the two MLPs ----
                h2 = mlp2(wd1, bd1, wd2, bd2, xT, "h")
                g2 = mlp2(wo1, bo1, wo2, bo2, xT, "g")

                # ---- diag = (relu(d + bdo) + dm) * x  (feature-major, fp32) ----
                psd = ps_acc.tile([N, BT], f32, tag="acc")
                for k in range(2):
                    nc.tensor.matmul(psd[:], wdo[:, k, :], h2[:, k],
                                     start=(k == 0), stop=(k == 1))
                dr = small_pool.tile([N, BT], f32, tag="dr")
                nc.scalar.activation(dr[:], psd[:], AF.Relu, bias=bdo[:, 0:1])
                dd = small_pool.tile([N, BT], f32, tag="dd")
                nc.vector.tensor_scalar_add(dd[:], dr[:], dm[:, 0:1])
                diag = small_pool.tile([N, BT], f32, tag="diag")
                nc.gpsimd.tensor_mul(out=diag[:], in0=dd[:], in1=xT[:])

                # ---- off = g2 @ Woo + boo, feature-major slices [128, 512] ----
                off = off_pool.tile([SL, NSLICES, BT], bf16, tag="off")
                for s in range(NSLICES):
                    pso = ps_off.tile([SL, BT], f32, tag="off")
                    for k in range(2):
                        nc.tensor.matmul(pso[:], woo[:, k, SL * s:SL * (s + 1)],
                                         g2[:, k], start=(k == 0), stop=(k == 1))
                    nc.scalar.add(off[:, s], pso[:], boo[:, s:s + 1])

                # ---- pass 1: v = Ecol^T (off * (Rrow xT)) + diag*x ----
                psv = ps_acc.tile([N, BT], f32, tag="acc")
                for s in range(NSLICES):
                    pse = ps_xe.tile([SL, BT], f32, tag="xe")
                    nc.tensor.matmul(pse[:], rrow[:, SL * s:SL * (s + 1)],
                                     xT[:], start=True, stop=True)
                    m1 = m_pool.tile([SL, BT], bf16, tag="m1")
                    nc.vector.tensor_mul(out=m1[:], in0=off[:, s], in1=pse[:])
                    nc.tensor.matmul(psv[:], ecol[:, N * s:N * (s + 1)],
                                     m1[:], start=(s == 0), stop=(s == NSLICES - 1))
                dvx = small_pool.tile([N, BT], f32, tag="dvx")
                nc.gpsimd.tensor_mul(out=dvx[:], in0=diag[:], in1=xT[:])
                v = small_pool.tile([N, BT], bf16, tag="v")
                nc.vector.tensor_add(out=v[:], in0=psv[:], in1=dvx[:])

                # ---- pass 2: out = Erow^T (off * (Rcol vT)) + diag*v ----
                pso2 = ps_acc.tile([N, BT], f32, tag="acc")
                for s in range(NSLICES):
                    pse = ps_xe.tile([SL, BT], f32, tag="xe")
                    nc.tensor.matmul(pse[:], rcol[:, SL * s:SL * (s + 1)],
                                     v[:], start=True, stop=True)
                    m2 = m_pool.tile([SL, BT], bf16, tag="m2")
                    nc.vector.tensor_mul(out=m2[:], in0=off[:, s], in1=pse[:])
                    nc.tensor.matmul(pso2[:], erow[:, N * s:N * (s + 1)],
                                     m2[:], start=(s == 0), stop=(s == NSLICES - 1))
                dvv = small_pool.tile([N, BT], f32, tag="dvv")
                nc.gpsimd.tensor_mul(out=dvv[:], in0=diag[:], in1=v[:])
                outf = small_pool.tile([N, BT], f32, tag="outf")
                nc.vector.tensor_add(out=outf[:], in0=pso2[:], in1=dvv[:])

                # ---- transpose back + store ----
                osb = out_pool.tile([128, 4, N], f32, tag="osb")
                for t in range(4):
                    psq = ps_xe.tile([128, N], f32, tag="xe")
                    nc.tensor.transpose(psq[:], outf[:, 128 * t:128 * (t + 1)],
                                        identf[:])
                    nc.scalar.copy(osb[:, t], psq[:])
                nc.sync.dma_start(out_view[:, 4 * b:4 * b + 4, :], osb[:])

    nc.compile()
    return nc


def _get_program():
    global _compiled
    if _compiled is None:
        _compiled = _build_program()
    return _compiled


def _host_consts(inputs):
    import ml_dtypes
    f = np.float32
    bf = ml_dtypes.bfloat16
    rows, cols = np.tril_indices(N, k=-1)         # length 2016
    # padded index arrays: entries p >= 2016 are dead (all matrices zero there)
    npad = OFFP - len(rows)                        # 32

    def onehot(idx, num, valid):
        m = np.zeros((num, OFFP), f)
        m[idx[valid], np.where(valid)[0]] = 1.0
        return m

    valid = np.ones(OFFP, bool)
    valid[len(rows):] = False
    rows_p = np.concatenate([rows, np.zeros(npad, int)])
    cols_p = np.concatenate([cols, np.zeros(npad, int)])

    rrow = onehot(rows_p, N, valid)               # [64, 2048]
    rcol = onehot(cols_p, N, valid)               # [64, 2048]
    ecol = np.zeros((SL, NSLICES, N), f)
    erow = np.zeros((SL, NSLICES, N), f)
    for s in range(NSLICES):
        for m in range(SL):
            p = SL * s + m
            if p < len(rows):
                ecol[m, s, cols[p]] = 1.0
                erow[m, s, rows[p]] = 1.0

    woo_pad = np.zeros((H, OFFP), f)
    woo_pad[:, :OFF] = np.asarray(inputs["Woo"], f)
    boo_pad = np.zeros(OFFP, f)
    boo_pad[:OFF] = np.asarray(inputs["boo"], f)

    def kt(w):  # [256, M] -> [128, 2, M]
        w = np.asarray(w, f)
        return np.ascontiguousarray(w.reshape(2, 128, -1).transpose(1, 0, 2))

    def bt(v):  # [256] -> [128, 2]
        return np.ascontiguousarray(np.asarray(v, f).reshape(2, 128).T)

    return {
        "wd1": np.asarray(inputs["Wd1"], f).astype(bf),
        "wd2": kt(inputs["Wd2"]).astype(bf),
        "wdo": kt(inputs["Wdo"]).astype(bf),
        "wo1": np.asarray(inputs["Wo1"], f).astype(bf),
        "wo2": kt(inputs["Wo2"]).astype(bf),
        "woo": kt(woo_pad).astype(bf),
        "bd1": bt(inputs["bd1"]),
        "bd2": bt(inputs["bd2"]),
        "bo1": bt(inputs["bo1"]),
        "bo2": bt(inputs["bo2"]),
        "bdo": np.asarray(inputs["bdo"], f).reshape(N, 1),
        "boo": np.ascontiguousarray(boo_pad.reshape(NSLICES, SL).T),
        "dm": np.asarray(inputs["damp_min"], f).reshape(N, 1),
        "rrow": rrow.astype(bf),
        "rcol": rcol.astype(bf),
        "ecol": np.ascontiguousarray(ecol.reshape(SL, NSLICES * N)).astype(bf),
        "erow": np.ascontiguousarray(erow.reshape(SL, NSLICES * N)).astype(bf),
    }


def kernel(trace=False, **inputs):
    from concourse.bass_utils import run_bass_kernel_spmd

    nc = _get_program()
    consts = _host_consts(inputs)
    x = np.ascontiguousarray(np.asarray(inputs["x"], np.float32))
    in_maps = [
        {"x": x[i * BLOCAL:(i + 1) * BLOCAL], **consts} for i in range(NCORES)
    ]
    res = run_bass_kernel_spmd(nc, in_maps, core_ids=list(range(NCORES)),
                               trace=trace)
    out = np.concatenate([res.results[i]["out"] for i in range(NCORES)], axis=0)
    if trace:
        kernel.last_results = res
    return out
